# revision 21
# baseline (speedup 1.0000x reference)
"""Trainium2 Bass kernel: causal multi-head attention with an extra time-mixing
matrix D (attn = D @ softmax(mask(Q K^T / sqrt(e))) @ V, concat heads, out proj).

Shapes (hardcoded): B=4, T=2048, d=1024, H=16, e=64, fp32.
Sharding over 8 NeuronCores: data-parallel over batch (4) x tensor-parallel over
heads (2 groups of 8). Each core computes its batch/head-group partial
y_part = concat(attn_heads) @ Wo_part^T; host sums the 2 partials per batch and
adds bo plus a rank-1 correction for the V bias (softmax rows sum to 1, so
P @ (V + 1 bv^T) = P@V + 1 bv^T commutes through D and Wo: the correction is
(D @ 1) x (Wo @ bv_concat), identical for every batch).

Projections run in float32r (hw hi/lo bf16 split, 1 cycle/row at moving>=256).
Everything downstream of the projections (scores operands, probs, V, PV, D,
out-proj operands) is bf16: matmuls then cost 1 cycle/row at ANY moving size,
which lets the diagonal score/PV windows be trimmed to their causal extent,
and D^T (bf16, 8.4MB) stays resident in SBUF so the D-matmul interleaves as
per-head-pair bursts inside the activation-bound attention phase.
"""

import sys

for _p in ("/opt/trn_rl_repo", "/root/.axon_site/_ro/trn_rl_repo"):
    if _p not in sys.path:
        sys.path.append(_p)

from contextlib import ExitStack

import numpy as np
import ml_dtypes

import concourse.bass as bass  # noqa: F401  (AP helpers)
import concourse.tile as tile
from concourse import bacc, mybir
from concourse.bass_utils import run_bass_kernel_spmd

dt = mybir.dt

B, T, D, H, E = 4, 2048, 1024, 16, 64
HG = 8          # heads per core (tensor-parallel group)
COEF = 1.0 / E ** 0.5
P = 128         # partitions
TQB = 512       # query-block width
NTQ = T // TQB  # 4 query blocks
NTC = T // P    # 16 time chunks
ND = D // P     # 8 contraction chunks (d)
EC = E + 1      # value cols + rowsum channel

_CACHED_NC = None


def _build_nc():
    """Build + compile the single-core program (same NEFF on all 8 cores)."""
    nc = bacc.Bacc("TRN2", target_bir_lowering=False, debug=False)
    f32, f32r, bf16 = dt.float32, dt.float32r, dt.bfloat16
    Exp = mybir.ActivationFunctionType.Exp
    Ident = mybir.ActivationFunctionType.Identity
    mult = mybir.AluOpType.mult

    xqT = nc.dram_tensor("xqT", [D, T], f32r, kind="ExternalInput").ap()
    xkT = nc.dram_tensor("xkT", [D, T], f32r, kind="ExternalInput").ap()
    xvT = nc.dram_tensor("xvT", [D, T], f32r, kind="ExternalInput").ap()
    wqT = nc.dram_tensor("wqT", [D, 512], f32r, kind="ExternalInput").ap()
    wkT = nc.dram_tensor("wkT", [D, 512], f32r, kind="ExternalInput").ap()
    wvT = nc.dram_tensor("wvT", [D, 512], f32r, kind="ExternalInput").ap()
    qkb = nc.dram_tensor("qkb", [P, 8], f32, kind="ExternalInput").ap()
    woT = nc.dram_tensor("woT", [512, D], bf16, kind="ExternalInput").ap()
    dTd = nc.dram_tensor("dTd", [T, T], bf16, kind="ExternalInput").ap()
    msk = nc.dram_tensor("msk", [P, 256], bf16, kind="ExternalInput").ap()
    idn = nc.dram_tensor("idn", [P, P], f32, kind="ExternalInput").ap()
    y = nc.dram_tensor("y", [T, D], f32, kind="ExternalOutput").ap()

    with tile.TileContext(nc) as tc, ExitStack() as ctx:
        # ---- persistent tiles ---------------------------------------------
        consts = ctx.enter_context(tc.tile_pool(name="consts", bufs=1))
        proj = ctx.enter_context(tc.tile_pool(name="proj", bufs=1))

        qt = [proj.tile([P, T], bf16, tag=f"qt{p}", name=f"qt{p}") for p in range(4)]
        kt = [proj.tile([P, T], bf16, tag=f"kt{p}", name=f"kt{p}") for p in range(4)]
        vt = [proj.tile([P, HG * EC], bf16, tag=f"vt{t}", name=f"vt{t}")
              for t in range(NTC)]

        # ---- projections ---------------------------------------------------
        def load_w(pool, wdram, name):
            tiles = []
            for d in range(ND):
                w = pool.tile([P, 512], f32r, tag=f"{name}{d}", name=f"{name}{d}")
                nc.sync.dma_start(w[:], wdram[P * d:P * (d + 1), :])
                tiles.append(w)
            return tiles

        def load_x_block(pool, xdram, tb, tag):
            xb = pool.tile([P, ND * 512], f32r, tag=tag, name=tag)
            for d in range(ND):
                nc.sync.dma_start(xb[:, 512 * d:512 * (d + 1)],
                                  xdram[P * d:P * (d + 1), TQB * tb:TQB * (tb + 1)])
            return xb

        xs_stack = ExitStack()
        xs_pool = xs_stack.enter_context(tc.tile_pool(name="xs", bufs=3))
        wqk_stack = ExitStack()
        wqk_pool = wqk_stack.enter_context(tc.tile_pool(name="wqk", bufs=1))

        # V projection: psum [t 128, 8 heads x 64] per t-chunk.
        # Weight DMAs for the NEXT phase are issued mid-phase so each phase
        # transition finds its weights already resident.
        with tc.tile_pool(name="wv", bufs=1) as wpool, \
             tc.tile_pool(name="psv", bufs=6, space="PSUM") as pspool:
            # interleave wv / first-x-block DMAs so matmul d can start as soon
            # as chunk d of both has landed
            wv = []
            xb0 = xs_pool.tile([P, ND * 512], f32r, tag="xs", name="xs")
            for d in range(ND):
                w = wpool.tile([P, 512], f32r, tag=f"wv{d}", name=f"wv{d}")
                nc.sync.dma_start(w[:], wvT[P * d:P * (d + 1), :])
                wv.append(w)
                nc.sync.dma_start(xb0[:, 512 * d:512 * (d + 1)],
                                  xvT[P * d:P * (d + 1), 0:TQB])
            qkbias = consts.tile([P, 8], f32, tag="qkbias")
            nc.sync.dma_start(qkbias[:], qkb[:])
            wq = None
            for tb in range(NTQ):
                xb = xb0 if tb == 0 else load_x_block(xs_pool, xvT, tb, "xs")
                if tb == 1:
                    wq = load_w(wqk_pool, wqT, "q")
                pss = [pspool.tile([P, 512], f32, tag="psv", name="psv")
                       for _ in range(4)]
                for d in range(ND):
                    for tc_ in range(4):
                        nc.tensor.matmul(
                            pss[tc_][:],
                            xb[:, 512 * d + P * tc_:512 * d + P * (tc_ + 1)],
                            wv[d][:],
                            start=(d == 0), stop=(d == ND - 1))
                for tc_ in range(4):
                    t = 4 * tb + tc_
                    dst = vt[t][:].rearrange("p (h c) -> p h c", c=EC)[:, :, 0:E]
                    src = pss[tc_][:].rearrange("p (h c) -> p h c", c=E)
                    nc.vector.tensor_copy(dst, src)
                    ones_dst = vt[t][:].rearrange("p (h c) -> p h c", c=EC)[:, :, E:EC]
                    nc.vector.memset(ones_dst, 1.0)

        # Q/K projections: psum [head-pair 128, t 512]; bias added during the
        # Act-engine psum evacuation (per-partition bias column).
        def qk_proj(xdram, wt, dest, name, bcol, mid_hook=None):
            with tc.tile_pool(name=f"ps{name}", bufs=6, space="PSUM") as pspool:
                for tb in range(NTQ):
                    xb = load_x_block(xs_pool, xdram, tb, "xs")
                    if tb == 1 and mid_hook is not None:
                        mid_hook()
                    pss = [pspool.tile([P, 512], f32, tag=f"ps{name}",
                                       name=f"ps{name}") for _ in range(4)]
                    for d in range(ND):
                        for p in range(4):
                            nc.tensor.matmul(
                                pss[p][:],
                                wt[d][:, P * p:P * (p + 1)],
                                xb[:, 512 * d:512 * (d + 1)],
                                start=(d == 0), stop=(d == ND - 1))
                    for p in range(4):
                        nc.scalar.activation(
                            dest[p][:, TQB * tb:TQB * (tb + 1)], pss[p][:],
                            Ident, bias=qkbias[:, bcol + p:bcol + p + 1])

        wk = []

        def load_wk():
            wk.extend(load_w(wqk_pool, wkT, "k"))

        qk_proj(xqT, wq, qt, "q", 0, mid_hook=load_wk)

        # small consts needed at attention start: before the bulky D^T loads
        idt = consts.tile([P, P], f32, tag="idt")
        nc.sync.dma_start(idt[:], idn[:])
        mskt = consts.tile([P, 256], bf16, tag="mskt")
        nc.sync.dma_start(mskt[:], msk[:])

        qk_proj(xkT, wk, kt, "k", 4)
        wqk_stack.close()
        xs_stack.close()

        # D^T resident loads: issued at attention start, consumed by bursts.
        # Pool opened only now — during projections its 64KB/partition would
        # not fit alongside the x-block and weight pools.
        dpool = ctx.enter_context(tc.tile_pool(name="dpool", bufs=1))
        dtt = [dpool.tile([P, T], bf16, tag=f"dt{t}", name=f"dt{t}")
               for t in range(NTC)]
        for t in range(NTC):
            nc.sync.dma_start(dtt[t][:], dTd[P * t:P * (t + 1), :])
        wo = []
        for cc in range(4):
            w = consts.tile([P, D], bf16, tag=f"wo{cc}", name=f"wo{cc}")
            nc.sync.dma_start(w[:], woT[P * cc:P * (cc + 1), :])
            wo.append(w)

        # ---- attention (scores^T -> exp -> mask -> PV^T + rowsums) with ----
        # ---- interleaved per-pair D-matmul bursts --------------------------
        # pvg split per pair-column: burst(p) reads only pair p's tiles, so a
        # pair's scale-writes never alias the previous pair's burst reads
        pvgp = ctx.enter_context(tc.tile_pool(name="pvg", bufs=1))
        pvg = [[pvgp.tile([P, P], bf16, tag=f"pvg{pp}_{t}", name=f"pvg{pp}_{t}")
                for t in range(NTC)] for pp in range(4)]
        a2sp = ctx.enter_context(tc.tile_pool(name="a2s", bufs=1))
        a2s = [a2sp.tile([P, 512], bf16, tag=f"a2s{i}", name=f"a2s{i}")
               for i in range(16)]

        sps = ctx.enter_context(tc.tile_pool(name="sps", bufs=2, space="PSUM"))
        pvps = ctx.enter_context(tc.tile_pool(name="pvps", bufs=1, space="PSUM"))
        flexps = ctx.enter_context(tc.tile_pool(name="flex", bufs=2, space="PSUM"))
        upool = ctx.enter_context(tc.tile_pool(name="upool", bufs=8))
        tmpp = ctx.enter_context(tc.tile_pool(name="tmpp", bufs=4))
        obp = ctx.enter_context(tc.tile_pool(name="obuf", bufs=4))

        mview = mskt[:].rearrange("p (b q) -> p b q", b=2)

        # Filler units: PE-dense work of the PREVIOUS pair's D-matmul burst,
        # emitted between attention chunks so the in-order PE queue has
        # independent work while Act computes exp / DVE applies masks.
        def burst_units(p):
            units = []
            state = {}
            for qb in range(NTQ):
                for g in range(4):      # 4 matmuls per unit
                    def mm(p=p, qb=qb, g=g):
                        if g == 0:
                            state[qb] = flexps.tile(
                                [P, 512], f32, tag="fx", name="a2")
                        a2 = state[qb]
                        for t in range(4 * g, 4 * g + 4):
                            nc.tensor.matmul(
                                a2[:], pvg[p][t][:],
                                dtt[t][:, TQB * qb:TQB * (qb + 1)],
                                start=(t == 0), stop=(t == NTC - 1))
                    units.append(mm)

                def evac(p=p, qb=qb):
                    nc.vector.tensor_copy(a2s[4 * p + qb][:], state[qb][:])
                units.append(evac)
            return units

        def emit_fill(fillers, n):
            for _ in range(n):
                if fillers:
                    fillers.pop(0)()

        def attention_pair(p, fillers):
            for i in range(NTQ):
                nch = 4 * (i + 1)
                pv = pvps.tile([P, 1024], f32, tag="pv")
                tmps = [tmpp.tile([P, 512], f32, tag="ntmp", name="ntmp")
                        for _ in range(2)]
                for c in range(nch):
                    j = c - (nch - 4)      # >=0 on diagonal chunks
                    lo = 128 * j if j > 0 else 0
                    sp = sps.tile([P, 1024], f32, tag="sp")
                    for h in range(2):     # row-packed pair: K=64 each
                        nc.tensor.matmul(
                            sp[:, 512 * h + lo:512 * (h + 1)],
                            kt[p][64 * h:64 * (h + 1), P * c:P * (c + 1)],
                            qt[p][64 * h:64 * (h + 1),
                                  TQB * i + lo:TQB * (i + 1)],
                            start=True, stop=True)
                    u = upool.tile([P, 1024], bf16, tag="u")
                    if lo:
                        spw = sp[:].rearrange("p (h q) -> p h q", h=2)[:, :, lo:512]
                        uw = u[:].rearrange("p (h q) -> p h q", h=2)[:, :, lo:512]
                        nc.scalar.activation(uw, spw, Exp, scale=COEF)
                    else:
                        nc.scalar.activation(u[:], sp[:], Exp, scale=COEF)
                    if j >= 0:             # triangular mask on the diag block
                        uw = u[:].rearrange("p (h q) -> p h q", h=2)[:, :, lo:lo + P]
                        nc.vector.tensor_tensor(uw, uw, mview, op=mult)
                    if c < 2:
                        emit_fill(fillers, 1)
                    for h in range(2):
                        nc.tensor.matmul(
                            pv[0:EC, 512 * h + lo:512 * (h + 1)],
                            vt[c][:, EC * (2 * p + h):EC * (2 * p + h + 1)],
                            u[:, 512 * h + lo:512 * (h + 1)],
                            start=(c == 0), stop=(c == nch - 1),
                            skip_group_check=True)
                    if j >= 0:
                        # columns [128j,128(j+1)) of pv are final (windows of
                        # later chunks start higher): evacuate them now so the
                        # pv slot is free the moment the block ends
                        for h in range(2):
                            nc.vector.tensor_copy(
                                tmps[h][0:E, P * j:P * (j + 1)],
                                pv[0:E, 512 * h + P * j:512 * h + P * (j + 1)])
                            nc.vector.reciprocal(
                                tmps[h][E:EC, P * j:P * (j + 1)],
                                pv[E:EC, 512 * h + P * j:512 * h + P * (j + 1)])
                    if c >= 2:
                        emit_fill(fillers, 1)
                # transpose into pvg (natural [t, head cols]).
                # tpk packs the 8 [128,65] transposes into one sp-pool slot;
                # slot k<7 sits at col 65k (bank A), k=7 at col 512 (bank B)
                # so no matmul write crosses a PSUM bank boundary.
                emit_fill(fillers, 1)
                tpk = sps.tile([P, 1024], f32, tag="sp", name="tpk")
                tcol = lambda k: 65 * k if k < 7 else 512
                for h in range(2):
                    for qs in range(4):
                        k = 4 * h + qs
                        nc.tensor.transpose(
                            tpk[0:P, tcol(k):tcol(k) + EC],
                            tmps[h][0:EC, P * qs:P * (qs + 1)],
                            idt[0:EC, 0:EC])
                for h in range(2):
                    for qs in range(4):
                        k = 4 * h + qs
                        nc.vector.tensor_scalar(
                            pvg[p][4 * i + qs][:, E * h:E * (h + 1)],
                            tpk[:, tcol(k):tcol(k) + E],
                            tpk[:, tcol(k) + E:tcol(k) + EC],
                            None, op0=mult)
            emit_fill(fillers, len(fillers))   # drain any leftovers

        fillers = []
        for p in range(4):
            attention_pair(p, fillers)
            fillers = burst_units(p)

        # ---- tail: last pair's burst interleaved with the out projection ---
        # y[q, :] = sum_cc a2s[cc, qb]^T @ wo[cc]
        for qb in range(NTQ):
            emit_fill(fillers, 5)          # burst(p=3, qb): 4 mm units + evac
            for qs in range(4):
                for nh in range(2):
                    op_ = flexps.tile([P, 512], f32, tag="fx", name="op")
                    for cc in range(4):
                        nc.tensor.matmul(
                            op_[:],
                            a2s[4 * cc + qb][:, P * qs:P * (qs + 1)],
                            wo[cc][:, 512 * nh:512 * (nh + 1)],
                            start=(cc == 0), stop=(cc == 3))
                    ob = obp.tile([P, 512], f32, tag="ob")
                    nc.scalar.copy(ob[:], op_[:])
                    nc.sync.dma_start(
                        y[TQB * qb + P * qs:TQB * qb + P * (qs + 1),
                          512 * nh:512 * (nh + 1)],
                        ob[:])

    nc.compile()
    return nc


def _prep_inputs(query_1, key_1, value_1, Wq, bq, Wk, bk, Wv, bv, Wo, bo, Dmat):
    """Host-side sharding: per-core input dicts."""
    f = np.float32
    bf = ml_dtypes.bfloat16

    def xT(x, b):
        return np.ascontiguousarray(np.asarray(x[b], f).T)

    # per head-group weights
    wqTs, wkTs, wvTs, woTs, qkbs = [], [], [], [], []
    for g in range(2):
        h0 = HG * g
        wq = np.zeros((D, 512), f)
        wk = np.zeros((D, 512), f)
        qkb = np.zeros((P, 8), f)
        for p in range(4):
            for h in range(2):
                hh = h0 + 2 * p + h
                c0 = 128 * p + 64 * h
                wq[:, c0:c0 + 64] = np.asarray(Wq[hh], f).T
                wk[:, c0:c0 + 64] = np.asarray(Wk[hh], f).T
                qkb[64 * h:64 * (h + 1), p] = np.asarray(bq[hh], f)
                qkb[64 * h:64 * (h + 1), 4 + p] = np.asarray(bk[hh], f)
        wv = np.zeros((D, 512), f)
        for jj in range(HG):
            wv[:, 64 * jj:64 * (jj + 1)] = np.asarray(Wv[h0 + jj], f).T
        wo = np.ascontiguousarray(
            np.asarray(Wo, f)[:, 64 * h0:64 * (h0 + HG)].T.astype(bf))
        wqTs.append(wq); wkTs.append(wk); wvTs.append(wv); woTs.append(wo)
        qkbs.append(qkb)

    dT = np.ascontiguousarray(np.asarray(Dmat, f).T.astype(bf))
    r = np.arange(P)[:, None]
    s = np.arange(P)[None, :]
    tri = (r <= s).astype(f)                             # [128, 128]
    msk = np.ascontiguousarray(np.tile(tri, (1, 2)).astype(bf))
    idn = np.eye(P, dtype=f)

    xqTs = [xT(query_1, b) for b in range(B)]
    xkTs = [xT(key_1, b) for b in range(B)]
    xvTs = [xT(value_1, b) for b in range(B)]

    in_maps = []
    for c in range(8):
        b, g = c // 2, c % 2
        in_maps.append({
            "xqT": xqTs[b], "xkT": xkTs[b], "xvT": xvTs[b],
            "wqT": wqTs[g], "wkT": wkTs[g], "wvT": wvTs[g],
            "qkb": qkbs[g], "woT": woTs[g],
            "dTd": dT, "msk": msk, "idn": idn,
        })
    return in_maps


def kernel(query_1, key_1, value_1, Wq, bq, Wk, bk, Wv, bv, Wo, bo, D):
    import os
    os.environ["BASS_NEVER_TRACE"] = "1"  # NTFF capture hangs over the axon relay
    global _CACHED_NC
    if _CACHED_NC is None:
        _CACHED_NC = _build_nc()
    nc = _CACHED_NC
    in_maps = _prep_inputs(query_1, key_1, value_1, Wq, bq, Wk, bk, Wv, bv, Wo, bo, D)
    res = run_bass_kernel_spmd(nc, in_maps, core_ids=list(range(8)))
    # host epilogue: sum head-group partials, add bo and the V-bias rank-1 term
    Wo_f = np.asarray(Wo, np.float32)
    bv_f = np.asarray(bv, np.float32).reshape(-1)        # concat over heads
    D_f = np.asarray(D, np.float32)
    corr = np.outer(D_f.sum(axis=1), Wo_f @ bv_f) + np.asarray(bo, np.float32)
    out = np.empty((B, T, 1024), np.float32)
    for b in range(B):
        out[b] = res.results[2 * b]["y"] + res.results[2 * b + 1]["y"] + corr
    return out


# revision 25
# speedup vs baseline: 1.0215x; 1.0215x over previous
"""Trainium2 Bass kernel: causal multi-head attention with an extra time-mixing
matrix D (attn = D @ softmax(mask(Q K^T / sqrt(e))) @ V, concat heads, out proj).

Shapes (hardcoded): B=4, T=2048, d=1024, H=16, e=64, fp32.
Sharding over 8 NeuronCores: data-parallel over batch (4) x tensor-parallel over
heads (2 groups of 8). Each core computes its batch/head-group partial
y_part = concat(attn_heads) @ Wo_part^T; host sums the 2 partials per batch and
adds bo plus a rank-1 correction for the V bias (softmax rows sum to 1, so
P @ (V + 1 bv^T) = P@V + 1 bv^T commutes through D and Wo: the correction is
(D @ 1) x (Wo @ bv_concat), identical for every batch).

Projections run in float32r (hw hi/lo bf16 split, 1 cycle/row at moving>=256).
Everything downstream of the projections (scores operands, probs, V, PV, D,
out-proj operands) is bf16: matmuls then cost 1 cycle/row at ANY moving size,
which lets the diagonal score/PV windows be trimmed to their causal extent,
and D^T (bf16, 8.4MB) stays resident in SBUF so the D-matmul interleaves as
per-head-pair bursts inside the activation-bound attention phase.
"""

import sys

for _p in ("/opt/trn_rl_repo", "/root/.axon_site/_ro/trn_rl_repo"):
    if _p not in sys.path:
        sys.path.append(_p)

from contextlib import ExitStack

import numpy as np
import ml_dtypes

import concourse.bass as bass  # noqa: F401  (AP helpers)
import concourse.tile as tile
from concourse import bacc, mybir
from concourse.bass_utils import run_bass_kernel_spmd

dt = mybir.dt

B, T, D, H, E = 4, 2048, 1024, 16, 64
HG = 8          # heads per core (tensor-parallel group)
COEF = 1.0 / E ** 0.5
P = 128         # partitions
TQB = 512       # query-block width
NTQ = T // TQB  # 4 query blocks
NTC = T // P    # 16 time chunks
ND = D // P     # 8 contraction chunks (d)
EC = E + 1      # value cols + rowsum channel

_CACHED_NC = None


def _build_nc():
    """Build + compile the single-core program (same NEFF on all 8 cores)."""
    nc = bacc.Bacc("TRN2", target_bir_lowering=False, debug=False)
    f32, f32r, bf16 = dt.float32, dt.float32r, dt.bfloat16
    Exp = mybir.ActivationFunctionType.Exp
    Ident = mybir.ActivationFunctionType.Identity
    mult = mybir.AluOpType.mult

    xqT = nc.dram_tensor("xqT", [D, T], bf16, kind="ExternalInput").ap()
    xkT = nc.dram_tensor("xkT", [D, T], bf16, kind="ExternalInput").ap()
    xvT = nc.dram_tensor("xvT", [D, T], bf16, kind="ExternalInput").ap()
    wqT = nc.dram_tensor("wqT", [D, 512], bf16, kind="ExternalInput").ap()
    wkT = nc.dram_tensor("wkT", [D, 512], bf16, kind="ExternalInput").ap()
    wvT = nc.dram_tensor("wvT", [D, 512], bf16, kind="ExternalInput").ap()
    qkb = nc.dram_tensor("qkb", [P, 8], f32, kind="ExternalInput").ap()
    woT = nc.dram_tensor("woT", [512, D], bf16, kind="ExternalInput").ap()
    dTd = nc.dram_tensor("dTd", [T, T], bf16, kind="ExternalInput").ap()
    msk = nc.dram_tensor("msk", [P, 256], bf16, kind="ExternalInput").ap()
    idn = nc.dram_tensor("idn", [P, P], f32, kind="ExternalInput").ap()
    y = nc.dram_tensor("y", [T, D], f32, kind="ExternalOutput").ap()

    with tile.TileContext(nc) as tc, ExitStack() as ctx:
        # ---- persistent tiles ---------------------------------------------
        consts = ctx.enter_context(tc.tile_pool(name="consts", bufs=1))
        proj = ctx.enter_context(tc.tile_pool(name="proj", bufs=1))

        qt = [proj.tile([P, T], bf16, tag=f"qt{p}", name=f"qt{p}") for p in range(4)]
        kt = [proj.tile([P, T], bf16, tag=f"kt{p}", name=f"kt{p}") for p in range(4)]
        vt = [proj.tile([P, HG * EC], bf16, tag=f"vt{t}", name=f"vt{t}")
              for t in range(NTC)]

        # ---- projections ---------------------------------------------------
        def load_w(pool, wdram, name):
            tiles = []
            for d in range(ND):
                w = pool.tile([P, 512], bf16, tag=f"{name}{d}", name=f"{name}{d}")
                nc.sync.dma_start(w[:], wdram[P * d:P * (d + 1), :])
                tiles.append(w)
            return tiles

        def load_x_block(pool, xdram, tb, tag):
            xb = pool.tile([P, ND * 512], bf16, tag=tag, name=tag)
            for d in range(ND):
                nc.sync.dma_start(xb[:, 512 * d:512 * (d + 1)],
                                  xdram[P * d:P * (d + 1), TQB * tb:TQB * (tb + 1)])
            return xb

        xs_stack = ExitStack()
        xs_pool = xs_stack.enter_context(tc.tile_pool(name="xs", bufs=3))
        wqk_stack = ExitStack()
        wqk_pool = wqk_stack.enter_context(tc.tile_pool(name="wqk", bufs=1))

        # V projection: psum [t 128, 8 heads x 64] per t-chunk.
        # Weight DMAs for the NEXT phase are issued mid-phase so each phase
        # transition finds its weights already resident.
        with tc.tile_pool(name="wv", bufs=1) as wpool, \
             tc.tile_pool(name="psv", bufs=6, space="PSUM") as pspool:
            # interleave wv / first-x-block DMAs so matmul d can start as soon
            # as chunk d of both has landed
            wv = []
            xb0 = xs_pool.tile([P, ND * 512], bf16, tag="xs", name="xs")
            for d in range(ND):
                w = wpool.tile([P, 512], bf16, tag=f"wv{d}", name=f"wv{d}")
                nc.sync.dma_start(w[:], wvT[P * d:P * (d + 1), :])
                wv.append(w)
                nc.sync.dma_start(xb0[:, 512 * d:512 * (d + 1)],
                                  xvT[P * d:P * (d + 1), 0:TQB])
            qkbias = consts.tile([P, 8], f32, tag="qkbias")
            nc.sync.dma_start(qkbias[:], qkb[:])
            wq = None
            for tb in range(NTQ):
                xb = xb0 if tb == 0 else load_x_block(xs_pool, xvT, tb, "xs")
                if tb == 1:
                    wq = load_w(wqk_pool, wqT, "q")
                pss = [pspool.tile([P, 512], f32, tag="psv", name="psv")
                       for _ in range(4)]
                for d in range(ND):
                    for tc_ in range(4):
                        nc.tensor.matmul(
                            pss[tc_][:],
                            xb[:, 512 * d + P * tc_:512 * d + P * (tc_ + 1)],
                            wv[d][:],
                            start=(d == 0), stop=(d == ND - 1))
                for tc_ in range(4):
                    t = 4 * tb + tc_
                    dst = vt[t][:].rearrange("p (h c) -> p h c", c=EC)[:, :, 0:E]
                    src = pss[tc_][:].rearrange("p (h c) -> p h c", c=E)
                    nc.vector.tensor_copy(dst, src)
                    ones_dst = vt[t][:].rearrange("p (h c) -> p h c", c=EC)[:, :, E:EC]
                    nc.vector.memset(ones_dst, 1.0)

        # Q/K projections: psum [head-pair 128, t 512]; bias added during the
        # Act-engine psum evacuation (per-partition bias column).
        def qk_proj(xdram, wt, dest, name, bcol, mid_hook=None):
            with tc.tile_pool(name=f"ps{name}", bufs=6, space="PSUM") as pspool:
                for tb in range(NTQ):
                    xb = load_x_block(xs_pool, xdram, tb, "xs")
                    if tb == 1 and mid_hook is not None:
                        mid_hook()
                    pss = [pspool.tile([P, 512], f32, tag=f"ps{name}",
                                       name=f"ps{name}") for _ in range(4)]
                    for d in range(ND):
                        for p in range(4):
                            nc.tensor.matmul(
                                pss[p][:],
                                wt[d][:, P * p:P * (p + 1)],
                                xb[:, 512 * d:512 * (d + 1)],
                                start=(d == 0), stop=(d == ND - 1))
                    for p in range(4):
                        nc.scalar.activation(
                            dest[p][:, TQB * tb:TQB * (tb + 1)], pss[p][:],
                            Ident, bias=qkbias[:, bcol + p:bcol + p + 1])

        wk = []

        def load_wk():
            wk.extend(load_w(wqk_pool, wkT, "k"))

        qk_proj(xqT, wq, qt, "q", 0, mid_hook=load_wk)

        # small consts needed at attention start: before the bulky D^T loads
        idt = consts.tile([P, P], f32, tag="idt")
        nc.sync.dma_start(idt[:], idn[:])
        mskt = consts.tile([P, 256], bf16, tag="mskt")
        nc.sync.dma_start(mskt[:], msk[:])

        qk_proj(xkT, wk, kt, "k", 4)
        wqk_stack.close()
        xs_stack.close()

        # D^T resident loads: issued at attention start, consumed by bursts.
        # Pool opened only now — during projections its 64KB/partition would
        # not fit alongside the x-block and weight pools.
        dpool = ctx.enter_context(tc.tile_pool(name="dpool", bufs=1))
        dtt = [dpool.tile([P, T], bf16, tag=f"dt{t}", name=f"dt{t}")
               for t in range(NTC)]
        for t in range(NTC):
            nc.sync.dma_start(dtt[t][:], dTd[P * t:P * (t + 1), :])
        wo = []
        for cc in range(4):
            w = consts.tile([P, D], bf16, tag=f"wo{cc}", name=f"wo{cc}")
            nc.sync.dma_start(w[:], woT[P * cc:P * (cc + 1), :])
            wo.append(w)

        # ---- attention (scores^T -> exp -> mask -> PV^T + rowsums) with ----
        # ---- interleaved per-pair D-matmul bursts --------------------------
        # pvg split per pair-column: burst(p) reads only pair p's tiles, so a
        # pair's scale-writes never alias the previous pair's burst reads
        pvgp = ctx.enter_context(tc.tile_pool(name="pvg", bufs=1))
        pvg = [[pvgp.tile([P, P], bf16, tag=f"pvg{pp}_{t}", name=f"pvg{pp}_{t}")
                for t in range(NTC)] for pp in range(4)]
        a2sp = ctx.enter_context(tc.tile_pool(name="a2s", bufs=1))
        a2s = [a2sp.tile([P, 512], bf16, tag=f"a2s{i}", name=f"a2s{i}")
               for i in range(16)]

        sps = ctx.enter_context(tc.tile_pool(name="sps", bufs=2, space="PSUM"))
        pvps = ctx.enter_context(tc.tile_pool(name="pvps", bufs=1, space="PSUM"))
        flexps = ctx.enter_context(tc.tile_pool(name="flex", bufs=2, space="PSUM"))
        upool = ctx.enter_context(tc.tile_pool(name="upool", bufs=8))
        tmpp = ctx.enter_context(tc.tile_pool(name="tmpp", bufs=4))
        obp = ctx.enter_context(tc.tile_pool(name="obuf", bufs=4))

        mview = mskt[:].rearrange("p (b q) -> p b q", b=2)

        # Filler units: PE-dense work of the PREVIOUS pair's D-matmul burst,
        # emitted between attention chunks so the in-order PE queue has
        # independent work while Act computes exp / DVE applies masks.
        def burst_units(p):
            units = []
            state = {}
            for qb in range(NTQ):
                for g in range(4):      # 4 matmuls per unit
                    def mm(p=p, qb=qb, g=g):
                        if g == 0:
                            state[qb] = flexps.tile(
                                [P, 512], f32, tag="fx", name="a2")
                        a2 = state[qb]
                        for t in range(4 * g, 4 * g + 4):
                            nc.tensor.matmul(
                                a2[:], pvg[p][t][:],
                                dtt[t][:, TQB * qb:TQB * (qb + 1)],
                                start=(t == 0), stop=(t == NTC - 1))
                    units.append(mm)

                def evac(p=p, qb=qb):
                    nc.vector.tensor_copy(a2s[4 * p + qb][:], state[qb][:])
                units.append(evac)
            return units

        def emit_fill(fillers, n):
            for _ in range(n):
                if fillers:
                    fillers.pop(0)()

        def attention_pair(p, fillers):
            for i in range(NTQ):
                nch = 4 * (i + 1)
                pv = pvps.tile([P, 1024], f32, tag="pv")
                tmps = [tmpp.tile([P, 512], f32, tag="ntmp", name="ntmp")
                        for _ in range(2)]
                for c in range(nch):
                    j = c - (nch - 4)      # >=0 on diagonal chunks
                    lo = 128 * j if j > 0 else 0
                    sp = sps.tile([P, 1024], f32, tag="sp")
                    for h in range(2):     # row-packed pair: K=64 each
                        nc.tensor.matmul(
                            sp[:, 512 * h + lo:512 * (h + 1)],
                            kt[p][64 * h:64 * (h + 1), P * c:P * (c + 1)],
                            qt[p][64 * h:64 * (h + 1),
                                  TQB * i + lo:TQB * (i + 1)],
                            start=True, stop=True)
                    u = upool.tile([P, 1024], bf16, tag="u")
                    if lo:
                        spw = sp[:].rearrange("p (h q) -> p h q", h=2)[:, :, lo:512]
                        uw = u[:].rearrange("p (h q) -> p h q", h=2)[:, :, lo:512]
                        nc.scalar.activation(uw, spw, Exp, scale=COEF)
                    else:
                        nc.scalar.activation(u[:], sp[:], Exp, scale=COEF)
                    if j >= 0:             # triangular mask on the diag block
                        uw = u[:].rearrange("p (h q) -> p h q", h=2)[:, :, lo:lo + P]
                        nc.vector.tensor_tensor(uw, uw, mview, op=mult)
                    if c < 2:
                        emit_fill(fillers, 1)
                    for h in range(2):
                        nc.tensor.matmul(
                            pv[0:EC, 512 * h + lo:512 * (h + 1)],
                            vt[c][:, EC * (2 * p + h):EC * (2 * p + h + 1)],
                            u[:, 512 * h + lo:512 * (h + 1)],
                            start=(c == 0), stop=(c == nch - 1),
                            skip_group_check=True)
                    if j >= 0:
                        # columns [128j,128(j+1)) of pv are final (windows of
                        # later chunks start higher): evacuate them now so the
                        # pv slot is free the moment the block ends
                        for h in range(2):
                            nc.vector.tensor_copy(
                                tmps[h][0:E, P * j:P * (j + 1)],
                                pv[0:E, 512 * h + P * j:512 * h + P * (j + 1)])
                            nc.vector.reciprocal(
                                tmps[h][E:EC, P * j:P * (j + 1)],
                                pv[E:EC, 512 * h + P * j:512 * h + P * (j + 1)])
                    if c >= 2:
                        emit_fill(fillers, 1)
                # transpose into pvg (natural [t, head cols]).
                # tpk packs the 8 [128,65] transposes into one sp-pool slot;
                # slot k<7 sits at col 65k (bank A), k=7 at col 512 (bank B)
                # so no matmul write crosses a PSUM bank boundary.
                emit_fill(fillers, 1)
                tpk = sps.tile([P, 1024], f32, tag="sp", name="tpk")
                tcol = lambda k: 65 * k if k < 7 else 512
                for h in range(2):
                    for qs in range(4):
                        k = 4 * h + qs
                        nc.tensor.transpose(
                            tpk[0:P, tcol(k):tcol(k) + EC],
                            tmps[h][0:EC, P * qs:P * (qs + 1)],
                            idt[0:EC, 0:EC])
                for h in range(2):
                    for qs in range(4):
                        k = 4 * h + qs
                        nc.vector.tensor_scalar(
                            pvg[p][4 * i + qs][:, E * h:E * (h + 1)],
                            tpk[:, tcol(k):tcol(k) + E],
                            tpk[:, tcol(k) + E:tcol(k) + EC],
                            None, op0=mult)
            emit_fill(fillers, len(fillers))   # drain any leftovers

        fillers = []
        for p in range(4):
            attention_pair(p, fillers)
            fillers = burst_units(p)

        # ---- tail: last pair's burst interleaved with the out projection ---
        # y[q, :] = sum_cc a2s[cc, qb]^T @ wo[cc]
        for qb in range(NTQ):
            emit_fill(fillers, 5)          # burst(p=3, qb): 4 mm units + evac
            for qs in range(4):
                for nh in range(2):
                    op_ = flexps.tile([P, 512], f32, tag="fx", name="op")
                    for cc in range(4):
                        nc.tensor.matmul(
                            op_[:],
                            a2s[4 * cc + qb][:, P * qs:P * (qs + 1)],
                            wo[cc][:, 512 * nh:512 * (nh + 1)],
                            start=(cc == 0), stop=(cc == 3))
                    ob = obp.tile([P, 512], f32, tag="ob")
                    nc.scalar.copy(ob[:], op_[:])
                    nc.sync.dma_start(
                        y[TQB * qb + P * qs:TQB * qb + P * (qs + 1),
                          512 * nh:512 * (nh + 1)],
                        ob[:])

    nc.compile()
    return nc


def _prep_inputs(query_1, key_1, value_1, Wq, bq, Wk, bk, Wv, bv, Wo, bo, Dmat):
    """Host-side sharding: per-core input dicts."""
    f = np.float32
    bf = ml_dtypes.bfloat16

    def xT(x, b):
        return np.ascontiguousarray(np.asarray(x[b], f).T.astype(bf))

    # per head-group weights
    wqTs, wkTs, wvTs, woTs, qkbs = [], [], [], [], []
    for g in range(2):
        h0 = HG * g
        wq = np.zeros((D, 512), f)
        wk = np.zeros((D, 512), f)
        qkb = np.zeros((P, 8), f)
        for p in range(4):
            for h in range(2):
                hh = h0 + 2 * p + h
                c0 = 128 * p + 64 * h
                wq[:, c0:c0 + 64] = np.asarray(Wq[hh], f).T
                wk[:, c0:c0 + 64] = np.asarray(Wk[hh], f).T
                qkb[64 * h:64 * (h + 1), p] = np.asarray(bq[hh], f)
                qkb[64 * h:64 * (h + 1), 4 + p] = np.asarray(bk[hh], f)
        wv = np.zeros((D, 512), f)
        for jj in range(HG):
            wv[:, 64 * jj:64 * (jj + 1)] = np.asarray(Wv[h0 + jj], f).T
        wo = np.ascontiguousarray(
            np.asarray(Wo, f)[:, 64 * h0:64 * (h0 + HG)].T.astype(bf))
        wqTs.append(wq.astype(bf)); wkTs.append(wk.astype(bf))
        wvTs.append(wv.astype(bf)); woTs.append(wo)
        qkbs.append(qkb)

    dT = np.ascontiguousarray(np.asarray(Dmat, f).T.astype(bf))
    r = np.arange(P)[:, None]
    s = np.arange(P)[None, :]
    tri = (r <= s).astype(f)                             # [128, 128]
    msk = np.ascontiguousarray(np.tile(tri, (1, 2)).astype(bf))
    idn = np.eye(P, dtype=f)

    xqTs = [xT(query_1, b) for b in range(B)]
    xkTs = [xT(key_1, b) for b in range(B)]
    xvTs = [xT(value_1, b) for b in range(B)]

    in_maps = []
    for c in range(8):
        b, g = c // 2, c % 2
        in_maps.append({
            "xqT": xqTs[b], "xkT": xkTs[b], "xvT": xvTs[b],
            "wqT": wqTs[g], "wkT": wkTs[g], "wvT": wvTs[g],
            "qkb": qkbs[g], "woT": woTs[g],
            "dTd": dT, "msk": msk, "idn": idn,
        })
    return in_maps


def kernel(query_1, key_1, value_1, Wq, bq, Wk, bk, Wv, bv, Wo, bo, D):
    import os
    os.environ["BASS_NEVER_TRACE"] = "1"  # NTFF capture hangs over the axon relay
    global _CACHED_NC
    if _CACHED_NC is None:
        _CACHED_NC = _build_nc()
    nc = _CACHED_NC
    in_maps = _prep_inputs(query_1, key_1, value_1, Wq, bq, Wk, bk, Wv, bv, Wo, bo, D)
    res = run_bass_kernel_spmd(nc, in_maps, core_ids=list(range(8)))
    # host epilogue: sum head-group partials, add bo and the V-bias rank-1 term
    Wo_f = np.asarray(Wo, np.float32)
    bv_f = np.asarray(bv, np.float32).reshape(-1)        # concat over heads
    D_f = np.asarray(D, np.float32)
    corr = np.outer(D_f.sum(axis=1), Wo_f @ bv_f) + np.asarray(bo, np.float32)
    out = np.empty((B, T, 1024), np.float32)
    for b in range(B):
        out[b] = res.results[2 * b]["y"] + res.results[2 * b + 1]["y"] + corr
    return out
